# revision 11
# baseline (speedup 1.0000x reference)
"""Trainium2 Bass kernel for MultiHeadSyntonicAttention.

Problem: B=2, S=2048, D=1024, H=16 heads, DH=64.
  q/k/v = Linear(query/key/value); per-head gnosis gate
  gn = sigmoid(k . wg + bg); scores = (q k^T / sqrt(dh)) * (1+gn);
  out = softmax(scores) v;  out = ((out Wo+bo) Wd+bd) Wh+bh.

Sharding (8 cores): core c -> batch b=c//4, head-group g=c%4 (4 heads).
Each core computes its heads' attention and a row-slice partial of the
fused output projection Wf = Wo@Wd@Wh; host sums 4 partials per batch.

Schedule design (v2): the Act engine is a hard serial floor (128 exp
instructions of [128,1024] ~ 154us measured) and the PE only reaches
its 2.4GHz p-state when continuously busy, so the kernel is built as a
gapless act-stream of exps fed by a PE stream that never idles:
  - pre-phase "chases" the input DMAs (QT[0] a0-half, KT[0], then the
    full V projection in 2 waves of 8 s-blocks) with dc-outer
    accumulation across psum banks; gates for heads 0,1 right after
    KT[0].
  - attention stream (h0a0,h1a0,h0a1,h1a1,h2a0,h3a0,h2a1,h3a1) emits
    QK -> exp(scale=(1+gn)/8 per k-row) -> lag-1 PV per k-block, and
    pops filler PE work between QK and PV: KT[1]+gates(2,3), QT chunks,
    and the fused output projection for q-blocks 0-7 near the end.
    When the filler queue is dry an ldweights-only warmup keeps the PE
    p-state hot (touches no psum).
  - biases ride the psum->sbuf casts (DVE tensor_scalar) instead of
    rank-1 matmuls; softmax normalization divides via
    reciprocal_approx_fast directly out of the ot psum; the gnosis
    sigmoid is 1/(1+e^-z) computed with the Exp table so the act engine
    never reloads tables mid-stream.
PSUM tags: st0(2) st1(2) ot(2) fv(1) fp(1) banks = 8.
"""

import sys

sys.path.insert(0, "/opt/trn_rl_repo")

import numpy as np
import ml_dtypes

BF16 = ml_dtypes.bfloat16

B, S, D, H = 2, 2048, 1024, 16
DH = D // H          # 64
HPC = 4              # heads per core
C = HPC * DH         # 256 head-local columns per core
NCORES = 8
ND = D // 128        # 8 d-chunks
NSB = S // 128       # 16 s-blocks
AW = 1024            # attention q-tile width (2 PSUM banks)
NA = S // AW         # 2
NPT = 6              # pt ring depth

_nc_cache = {}


def build_bass():
    import concourse.bass as bass
    import concourse.mybir as mybir
    import concourse.tile as tile
    from concourse import bacc

    f32 = mybir.dt.float32
    bf16 = mybir.dt.bfloat16
    Alu = mybir.AluOpType
    Act = mybir.ActivationFunctionType

    nc = bacc.Bacc(None, target_bir_lowering=False, name="syntonic_attn")

    qT_d = nc.dram_tensor("qT", [D, S], bf16, kind="ExternalInput")
    kT_d = nc.dram_tensor("kT", [D, S], bf16, kind="ExternalInput")
    vT_d = nc.dram_tensor("vT", [D, S], bf16, kind="ExternalInput")
    # weights pre-packed host-side into tile layout
    wq_d = nc.dram_tensor("wqp", [128, ND * C], bf16, kind="ExternalInput")
    wk_d = nc.dram_tensor("wkp", [128, ND * C], bf16, kind="ExternalInput")
    wv_d = nc.dram_tensor("wvp", [128, ND * C], bf16, kind="ExternalInput")
    wf_d = nc.dram_tensor("wfp", [128, 2 * D], bf16, kind="ExternalInput")
    wg_d = nc.dram_tensor("wgp", [128, 2 * 2], bf16, kind="ExternalInput")
    bq_d = nc.dram_tensor("bqc", [128, 2], f32, kind="ExternalInput")
    bk_d = nc.dram_tensor("bkc", [128, 2], f32, kind="ExternalInput")
    bv_d = nc.dram_tensor("vb4", [128, C], f32, kind="ExternalInput")
    bg_d = nc.dram_tensor("bgneg", [128, 1], f32, kind="ExternalInput")
    out_d = nc.dram_tensor("out", [S, D], bf16, kind="ExternalOutput")

    with tile.TileContext(nc) as tc:
        with (
            tc.tile_pool(name="res", bufs=1) as res,
            tc.tile_pool(name="acts", bufs=1) as acts,
            tc.tile_pool(name="work", bufs=2) as work,
            tc.tile_pool(name="outp", bufs=3) as outp,
            tc.tile_pool(name="psum", bufs=1, space="PSUM") as psum,
        ):
            # ---------------- resident input tiles ----------------
            qT = [res.tile([128, S], bf16, tag=f"qT{i}", name=f"qT{i}") for i in range(ND)]
            kT = [res.tile([128, S], bf16, tag=f"kT{i}", name=f"kT{i}") for i in range(ND)]
            vT = [res.tile([128, S], bf16, tag=f"vT{i}", name=f"vT{i}") for i in range(ND)]
            wq = res.tile([128, ND * C], bf16, tag="wq")
            wk = res.tile([128, ND * C], bf16, tag="wk")
            wv = res.tile([128, ND * C], bf16, tag="wv")
            wf = res.tile([128, 2 * D], bf16, tag="wf")
            wg4 = res.tile([128, 4], bf16, tag="wg4")
            bqc = res.tile([128, 2], f32, tag="bqc")
            bkc = res.tile([128, 2], f32, tag="bkc")
            vb4 = res.tile([128, C], f32, tag="vb4")
            bgneg = res.tile([128, 1], f32, tag="bgneg")

            def wsl(w, dc, cb):
                return w[:, dc * C + cb * 128: dc * C + (cb + 1) * 128]

            # ---------------- DMA emission (priority order) ----------------
            nc.sync.dma_start(wq[:], wq_d[:])
            nc.sync.dma_start(wk[:], wk_d[:])
            nc.sync.dma_start(wg4[:], wg_d[:])
            nc.sync.dma_start(bqc[:], bq_d[:])
            nc.sync.dma_start(bkc[:], bk_d[:])
            nc.sync.dma_start(bgneg[:], bg_d[:])
            for i in range(ND):   # qT a0 halves first (QT[0]a0 chase)
                nc.sync.dma_start(qT[i][:, 0:AW], qT_d[i * 128:(i + 1) * 128, 0:AW])
            for i in range(ND):   # kT full (KT[0] chase)
                nc.sync.dma_start(kT[i][:], kT_d[i * 128:(i + 1) * 128, :])
            nc.sync.dma_start(wv[:], wv_d[:])
            nc.sync.dma_start(vb4[:], bv_d[:])
            for i in range(ND):   # vT (V projection waves chase this)
                nc.sync.dma_start(vT[i][:], vT_d[i * 128:(i + 1) * 128, :])
            nc.sync.dma_start(wf[:], wf_d[:])
            for i in range(ND):   # qT a1 halves
                nc.sync.dma_start(qT[i][:, AW:S], qT_d[i * 128:(i + 1) * 128, AW:S])

            # ---------------- activation / state tiles ----------------
            QT = [acts.tile([128, S], bf16, tag=f"QT{i}", name=f"QT{i}") for i in range(2)]
            KT = [acts.tile([128, S], bf16, tag=f"KT{i}", name=f"KT{i}") for i in range(2)]
            ctxT = [acts.tile([128, S], bf16, tag=f"ctxT{i}", name=f"ctxT{i}")
                    for i in range(2)]
            gscT = [acts.tile([128, HPC], f32, tag=f"gsc{i}", name=f"gsc{i}")
                    for i in range(NSB)]
            V = [acts.tile([128, HPC * 2 * DH], bf16, tag=f"V{i}", name=f"V{i}")
                 for i in range(NSB)]

            # ---------------- pre-phase: chase DMAs ----------------
            # QT[0] a0-half: dc-outer over 2 psum chunks (tags st0, st1)
            qps = [psum.tile([128, 512], f32, tag=f"st{ch}", name=f"qps{ch}")
                   for ch in range(2)]
            for dc in range(ND):
                for ch in range(2):
                    nc.tensor.matmul(
                        qps[ch][:], wsl(wq, dc, 0), qT[dc][:, ch * 512:(ch + 1) * 512],
                        start=(dc == 0), stop=(dc == ND - 1), skip_group_check=True)
            for ch in range(2):
                nc.vector.tensor_scalar_add(
                    QT[0][:, ch * 512:(ch + 1) * 512], qps[ch][:], bqc[:, 0:1])

            # KT[0]: dc-outer over 4 psum chunks (tags st0, st1, ot, fv)
            ktags = ["st0", "st1", "ot", "fv"]
            kps = [psum.tile([128, 512], f32, tag=ktags[a], name=f"kps{a}")
                   for a in range(4)]
            for dc in range(ND):
                for a in range(4):
                    nc.tensor.matmul(
                        kps[a][:], wsl(wk, dc, 0), kT[dc][:, a * 512:(a + 1) * 512],
                        start=(dc == 0), stop=(dc == ND - 1), skip_group_check=True)
            for a in range(4):
                nc.vector.tensor_scalar_add(
                    KT[0][:, a * 512:(a + 1) * 512], kps[a][:], bkc[:, 0:1])

            # gates: gsc = (1 + 1/(1+e^-z))/8, z = K.wg + bg (per head, k-token)
            def emit_gates(sb, cb):
                gps = psum.tile([128, 2], f32, tag="fp", name=f"gps{cb}_{sb}")
                nc.tensor.matmul(
                    gps[:], KT[cb][:, sb * 128:(sb + 1) * 128],
                    wg4[:, 2 * cb: 2 * cb + 2],
                    start=True, stop=True, skip_group_check=True)
                en = work.tile([128, 2], f32, tag="gen", name=f"gen{cb}_{sb}", bufs=2)
                nc.scalar.activation(en[:], gps[:], Act.Exp, bias=bgneg[:], scale=-1.0)
                u = work.tile([128, 2], f32, tag="gu", name=f"gu{cb}_{sb}", bufs=2)
                nc.vector.tensor_scalar_add(u[:], en[:], 1.0)
                r = work.tile([128, 2], f32, tag="gr", name=f"gr{cb}_{sb}", bufs=2)
                nc.vector.reciprocal(r[:], u[:])
                nc.vector.tensor_scalar(
                    gscT[sb][:, 2 * cb: 2 * cb + 2], r[:], 1.0, 0.125,
                    Alu.add, Alu.mult)

            for sb in range(NSB):
                emit_gates(sb, 0)

            # KT[1] (a-serial on fp) + gates for heads 2,3 + QT[1]a0:
            # bridges the PE between KT[0] and the vT DMA landing.
            def proj_chunk_now(w, cbw, x, dest, bias, a):
                ps = psum.tile([128, 512], f32, tag="fp", name=f"pp{id(w)}_{a}")
                for dc in range(ND):
                    nc.tensor.matmul(
                        ps[:], wsl(w, dc, cbw), x[dc][:, a * 512:(a + 1) * 512],
                        start=(dc == 0), stop=(dc == ND - 1), skip_group_check=True)
                nc.vector.tensor_scalar_add(
                    dest[:, a * 512:(a + 1) * 512], ps[:], bias)

            for a in range(4):
                proj_chunk_now(wk, 1, kT, KT[1], bkc[:, 1:2], a)
                for sb in range(4 * a, 4 * a + 4):
                    emit_gates(sb, 1)
            for ch in range(2):
                proj_chunk_now(wq, 1, qT, QT[1], bqc[:, 1:2], ch)

            # bridge dummies: keep the PE p-state hot until vT lands
            for _ in range(40):
                wd = psum.tile([128, 128], f32, tag="fp", name="wdp")
                nc.tensor.matmul(wd[:], wq[:, 0:128], wq[:, 128:256],
                                 start=True, stop=True, skip_group_check=True)

            # V projection: 4 waves of 4 s-blocks, dc-outer across 4 psum
            # banks (psum start=True zeroes whole 2KB banks, so one
            # accumulation group per bank), chasing the vT DMAs.
            for wave in range(4):
                vps = [psum.tile([128, C], f32, tag=ktags[i], name=f"vps{wave}{i}")
                       for i in range(4)]
                for dc in range(ND):
                    for i in range(4):
                        sb = wave * 4 + i
                        nc.tensor.matmul(
                            vps[i][:],
                            vT[dc][:, sb * 128:(sb + 1) * 128],
                            wv[:, dc * C:(dc + 1) * C],
                            start=(dc == 0), stop=(dc == ND - 1),
                            skip_group_check=True)
                for i in range(4):
                    sb = wave * 4 + i
                    vv = V[sb][:].rearrange("p (h x) -> p h x", h=HPC)
                    nc.vector.memset(vv[:, :, DH:2 * DH], 1.0)
                    nc.vector.tensor_tensor(
                        vv[:, :, 0:DH],
                        vps[i][:].rearrange("p (h x) -> p h x", h=HPC),
                        vb4[:].rearrange("p (h x) -> p h x", h=HPC),
                        Alu.add)

            # ---------------- filler units ----------------
            fillers = []   # (cost_cycles, not_before_iter, closure)

            def proj_chunk_units(w, cbw, x, dest, bias, a, nb):
                # one 512-wide projection chunk as 2 units (4 mms each + cast)
                qs = slice(a * 512, (a + 1) * 512)
                ps_box = [None]

                def half(first):
                    if first:
                        ps_box[0] = psum.tile([128, 512], f32, tag="fp",
                                              name=f"fps{len(fillers)}_{a}")
                    for dc in (range(0, 4) if first else range(4, ND)):
                        nc.tensor.matmul(
                            ps_box[0][:], wsl(w, dc, cbw), x[dc][:, qs],
                            start=(dc == 0), stop=(dc == ND - 1),
                            skip_group_check=True)
                    if not first:
                        nc.vector.tensor_scalar_add(dest[:, qs], ps_box[0][:], bias)

                fillers.append((2048, nb, lambda: half(True)))
                fillers.append((2048, nb, lambda: half(False)))

            def outproj_unit(qb, oc, ob, nb):
                def run():
                    ps = psum.tile([128, 512], f32, tag="fp", name=f"ops{qb}_{oc}")
                    for cc in range(2):
                        nc.tensor.matmul(
                            ps[:], ctxT[cc][:, qb * 128:(qb + 1) * 128],
                            wf[:, cc * D + oc * 512: cc * D + (oc + 1) * 512],
                            start=(cc == 0), stop=(cc == 1),
                            skip_group_check=True)
                    nc.vector.tensor_copy(ob[:, oc * 512:(oc + 1) * 512], ps[:])
                    if oc == 1:
                        nc.sync.dma_start(out_d[qb * 128:(qb + 1) * 128, :], ob[:])

                fillers.append((1024, nb, run))

            # QT[0] a1 and QT[1] a1 (qT a1-halves resident by stream start)
            for ch in range(2, 4):
                proj_chunk_units(wq, 0, qT, QT[0], bqc[:, 0:1], ch, 0)
            for ch in range(2, 4):
                proj_chunk_units(wq, 1, qT, QT[1], bqc[:, 1:2], ch, 0)
            # out-projection first q-half (ctxT a0 complete after iter 96)
            obs = {}
            for qb in range(8):
                obs[qb] = outp.tile([128, D], bf16, tag="ob", name=f"ob{qb}")
                outproj_unit(qb, 0, obs[qb], 98)
                outproj_unit(qb, 1, obs[qb], 98)

            # ---------------- attention stream ----------------
            blocks = [(0, 0), (1, 0), (0, 1), (1, 1), (2, 0), (3, 0), (2, 1), (3, 1)]
            budget = [0.0]
            it = [0]

            def pop_fillers():
                popped = 0
                idx = 0
                while idx < len(fillers) and popped < 2:
                    cost, nb, fn = fillers[idx]
                    if it[0] >= nb and budget[0] >= cost:
                        fillers.pop(idx)
                        fn()
                        budget[0] -= cost
                        popped += 1
                        idx = 0
                    elif it[0] >= nb:
                        break   # head eligible but budget exhausted
                    else:
                        idx += 1
                return popped

            def dummy_warm():
                wd = psum.tile([128, 128], f32, tag="fv", name="warm")
                nc.tensor.matmul(wd[:], wq[:, 0:128], wq[:, 128:256],
                                 start=True, stop=True, skip_group_check=True)

            pts = {}
            for h, a in blocks:
                cb, po = h // 2, (h % 2) * 64
                vsl = slice(h * 2 * DH, (h + 1) * 2 * DH)
                ot = psum.tile([128, AW], f32, tag="ot", name=f"ot{h}{a}")
                for kb in range(NSB):
                    st = psum.tile([128, AW], f32, tag=f"st{kb % 2}",
                                   name=f"st{h}{a}{kb}")
                    for qc in range(2):
                        nc.tensor.matmul(
                            st[:, qc * 512:(qc + 1) * 512],
                            KT[cb][po:po + 64, kb * 128:(kb + 1) * 128],
                            QT[cb][po:po + 64, a * AW + qc * 512: a * AW + (qc + 1) * 512],
                            start=True, stop=True)
                    pt = work.tile([128, AW], bf16, tag="pt",
                                   name=f"pt{h}{a}{kb}", bufs=NPT)
                    nc.scalar.activation(pt[:], st[:], Act.Exp,
                                         scale=gscT[kb][:, h:h + 1])
                    pts[(h, a, kb)] = pt
                    budget[0] = min(budget[0] + 700, 2200)
                    n = pop_fillers()
                    for _ in range(1 if n else 2):
                        dummy_warm()
                    if kb > 0:
                        ptp = pts.pop((h, a, kb - 1))
                        for qc in range(2):
                            nc.tensor.matmul(
                                ot[:, qc * 512:(qc + 1) * 512], V[kb - 1][:, vsl],
                                ptp[:, qc * 512:(qc + 1) * 512],
                                start=(kb == 1), stop=False, skip_group_check=True)
                    it[0] += 1
                ptp = pts.pop((h, a, NSB - 1))
                for qc in range(2):
                    nc.tensor.matmul(
                        ot[:, qc * 512:(qc + 1) * 512], V[NSB - 1][:, vsl],
                        ptp[:, qc * 512:(qc + 1) * 512],
                        start=False, stop=True, skip_group_check=True)
                # copy out of psum, then normalize on SBUF
                otc = work.tile([128, AW], f32, tag="otc", name=f"otc{h}{a}", bufs=2)
                nc.vector.tensor_copy(otc[:], ot[:])
                rec = work.tile([DH, AW], f32, tag="rec", name=f"rec{h}{a}", bufs=2)
                nc.vector.reciprocal(rec[:], otc[DH:2 * DH, :])
                nc.vector.tensor_tensor(
                    ctxT[cb][po:po + 64, a * AW:(a + 1) * AW],
                    otc[0:DH, :], rec[:], Alu.mult)

            # drain remaining fillers (out-proj qb0-7 stragglers)
            budget[0] = 1e9
            it[0] = 10 ** 6
            while fillers:
                if pop_fillers() == 0:
                    break

            # ---------------- tail: out-projection q-half 2 ----------------
            for qb in range(8, NSB):
                ob = outp.tile([128, D], bf16, tag="ob", name=f"ob{qb}")
                for oc in range(2):
                    ps = psum.tile([128, 512], f32, tag=("fp", "fv")[oc],
                                   name=f"tps{qb}_{oc}")
                    for cc in range(2):
                        nc.tensor.matmul(
                            ps[:], ctxT[cc][:, qb * 128:(qb + 1) * 128],
                            wf[:, cc * D + oc * 512: cc * D + (oc + 1) * 512],
                            start=(cc == 0), stop=(cc == 1),
                            skip_group_check=True)
                    if oc == 0:
                        nc.vector.tensor_copy(ob[:, 0:512], ps[:])
                    else:
                        nc.scalar.activation(ob[:, 512:1024], ps[:], Act.Copy)
                nc.sync.dma_start(out_d[qb * 128:(qb + 1) * 128, :], ob[:])

    nc.finalize()
    return nc


def get_nc():
    if "nc" not in _nc_cache:
        _nc_cache["nc"] = build_bass()
    return _nc_cache["nc"]


def make_in_maps(query, key_, value, Wq, bq, Wk, bk, Wv, bv, wg, bg, Wo, bo, Wd, bd, Wh, bh):
    """Host-side sharding: returns (in_maps for 8 cores, fused bias)."""
    f = np.asarray
    Wf = f(Wo, np.float64) @ f(Wd, np.float64) @ f(Wh, np.float64)
    bf = (f(bo, np.float64) @ f(Wd, np.float64) @ f(Wh, np.float64)
          + f(bd, np.float64) @ f(Wh, np.float64) + f(bh, np.float64))

    # gnosis weight packed per cb-chunk: col cb*2+hh has wg in rows of
    # head hh within chunk cb
    wgp = np.zeros((128, 4), np.float32)
    wgf = np.asarray(wg, np.float32)
    for cbi in range(2):
        for hh in range(2):
            wgp[hh * 64:(hh + 1) * 64, cbi * 2 + hh] = wgf
    wgp = wgp.astype(BF16)
    bgneg = np.full((128, 1), -np.float32(bg), np.float32)

    def pack_w(Wm, cols):
        # [D, 256] -> [128, ND*256]: w[p, dc*256+c] = Wm[dc*128+p, c]
        Wc = np.asarray(Wm, np.float32)[:, cols]
        return np.ascontiguousarray(
            Wc.reshape(ND, 128, C).transpose(1, 0, 2).reshape(128, ND * C)
        ).astype(BF16)

    xT = []
    for b in range(B):
        xT.append(tuple(
            np.ascontiguousarray(np.asarray(x[b], np.float32).T).astype(BF16)
            for x in (query, key_, value)
        ))

    in_maps = []
    for c in range(NCORES):
        b, g = divmod(c, HPC)
        cols = slice(g * C, (g + 1) * C)
        qTb, kTb, vTb = xT[b]
        wfp = np.ascontiguousarray(Wf[cols, :]).astype(np.float32)
        wfp = np.ascontiguousarray(
            wfp.reshape(2, 128, D).transpose(1, 0, 2).reshape(128, 2 * D)
        ).astype(BF16)
        bqcol = np.zeros((128, 2), np.float32)
        bkcol = np.zeros((128, 2), np.float32)
        for cbi in range(2):
            bqcol[:, cbi] = np.asarray(bq, np.float32)[g * C + cbi * 128:
                                                       g * C + (cbi + 1) * 128]
            bkcol[:, cbi] = np.asarray(bk, np.float32)[g * C + cbi * 128:
                                                       g * C + (cbi + 1) * 128]
        vb4 = np.broadcast_to(
            np.asarray(bv, np.float32)[cols][None, :], (128, C)).copy()
        in_maps.append({
            "qT": qTb, "kT": kTb, "vT": vTb,
            "wqp": pack_w(Wq, cols),
            "wkp": pack_w(Wk, cols),
            "wvp": pack_w(Wv, cols),
            "wfp": wfp,
            "wgp": wgp, "bgneg": bgneg,
            "bqc": bqcol, "bkc": bkcol, "vb4": vb4,
        })
    return in_maps, bf.astype(np.float32)


def gather(results, bf):
    out = np.zeros((B, S, D), np.float32)
    for c in range(NCORES):
        b = c // HPC
        out[b] += np.asarray(results[c]["out"], np.float32)
    out += bf[None, None, :]
    return out


def kernel(**inputs):
    from concourse.bass_utils import run_bass_kernel_spmd

    nc = get_nc()
    in_maps, bf = make_in_maps(**inputs)
    res = run_bass_kernel_spmd(nc, in_maps, core_ids=list(range(NCORES)))
    return gather(res.results, bf)


# revision 12
# speedup vs baseline: 1.1109x; 1.1109x over previous
"""Trainium2 Bass kernel for MultiHeadSyntonicAttention.

Problem: B=2, S=2048, D=1024, H=16 heads, DH=64.
  q/k/v = Linear(query/key/value); per-head gnosis gate
  gn = sigmoid(k . wg + bg); scores = (q k^T / sqrt(dh)) * (1+gn);
  out = softmax(scores) v;  out = ((out Wo+bo) Wd+bd) Wh+bh.

Sharding (8 cores): core c -> batch b=c//4, head-group g=c%4 (4 heads).
Each core computes its heads' attention and a row-slice partial of the
fused output projection Wf = Wo@Wd@Wh; host sums 4 partials per batch.

Schedule design (v2): the Act engine is a hard serial floor (128 exp
instructions of [128,1024] ~ 154us measured) and the PE only reaches
its 2.4GHz p-state when continuously busy, so the kernel is built as a
gapless act-stream of exps fed by a PE stream that never idles:
  - pre-phase "chases" the input DMAs (QT[0] a0-half, KT[0], then the
    full V projection in 2 waves of 8 s-blocks) with dc-outer
    accumulation across psum banks; gates for heads 0,1 right after
    KT[0].
  - attention stream (h0a0,h1a0,h0a1,h1a1,h2a0,h3a0,h2a1,h3a1) emits
    QK -> exp(scale=(1+gn)/8 per k-row) -> lag-1 PV per k-block, and
    pops filler PE work between QK and PV: KT[1]+gates(2,3), QT chunks,
    and the fused output projection for q-blocks 0-7 near the end.
    When the filler queue is dry an ldweights-only warmup keeps the PE
    p-state hot (touches no psum).
  - biases ride the psum->sbuf casts (DVE tensor_scalar) instead of
    rank-1 matmuls; softmax normalization divides via
    reciprocal_approx_fast directly out of the ot psum; the gnosis
    sigmoid is 1/(1+e^-z) computed with the Exp table so the act engine
    never reloads tables mid-stream.
PSUM tags: st0(2) st1(2) ot(2) fv(1) fp(1) banks = 8.
"""

import sys

sys.path.insert(0, "/opt/trn_rl_repo")

import numpy as np
import ml_dtypes

BF16 = ml_dtypes.bfloat16

B, S, D, H = 2, 2048, 1024, 16
DH = D // H          # 64
HPC = 4              # heads per core
C = HPC * DH         # 256 head-local columns per core
NCORES = 8
ND = D // 128        # 8 d-chunks
NSB = S // 128       # 16 s-blocks
AW = 1024            # attention q-tile width (2 PSUM banks)
NA = S // AW         # 2
NPT = 6              # pt ring depth

_nc_cache = {}


def build_bass():
    import concourse.bass as bass
    import concourse.mybir as mybir
    import concourse.tile as tile
    from concourse import bacc

    f32 = mybir.dt.float32
    bf16 = mybir.dt.bfloat16
    Alu = mybir.AluOpType
    Act = mybir.ActivationFunctionType

    nc = bacc.Bacc(None, target_bir_lowering=False, name="syntonic_attn")

    qT_d = nc.dram_tensor("qT", [D, S], bf16, kind="ExternalInput")
    kT_d = nc.dram_tensor("kT", [D, S], bf16, kind="ExternalInput")
    vT_d = nc.dram_tensor("vT", [D, S], bf16, kind="ExternalInput")
    # weights pre-packed host-side into tile layout
    wq_d = nc.dram_tensor("wqp", [128, ND * C], bf16, kind="ExternalInput")
    wk_d = nc.dram_tensor("wkp", [128, ND * C], bf16, kind="ExternalInput")
    wv_d = nc.dram_tensor("wvp", [128, ND * C], bf16, kind="ExternalInput")
    wf_d = nc.dram_tensor("wfp", [128, 2 * D], bf16, kind="ExternalInput")
    wg_d = nc.dram_tensor("wgp", [128, 2 * 2], bf16, kind="ExternalInput")
    bq_d = nc.dram_tensor("bqc", [128, 2], f32, kind="ExternalInput")
    bk_d = nc.dram_tensor("bkc", [128, 2], f32, kind="ExternalInput")
    bv_d = nc.dram_tensor("vb4", [128, C], f32, kind="ExternalInput")
    bg_d = nc.dram_tensor("bgneg", [128, 1], f32, kind="ExternalInput")
    out_d = nc.dram_tensor("out", [S, D], bf16, kind="ExternalOutput")

    with tile.TileContext(nc) as tc:
        with (
            tc.tile_pool(name="res", bufs=1) as res,
            tc.tile_pool(name="acts", bufs=1) as acts,
            tc.tile_pool(name="work", bufs=2) as work,
            tc.tile_pool(name="outp", bufs=3) as outp,
            tc.tile_pool(name="psum", bufs=1, space="PSUM") as psum,
        ):
            # ---------------- resident input tiles ----------------
            qT = [res.tile([128, S], bf16, tag=f"qT{i}", name=f"qT{i}") for i in range(ND)]
            kT = [res.tile([128, S], bf16, tag=f"kT{i}", name=f"kT{i}") for i in range(ND)]
            vT = [res.tile([128, S], bf16, tag=f"vT{i}", name=f"vT{i}") for i in range(ND)]
            wq = res.tile([128, ND * C], bf16, tag="wq")
            wk = res.tile([128, ND * C], bf16, tag="wk")
            wv = res.tile([128, ND * C], bf16, tag="wv")
            wf = res.tile([128, 2 * D], bf16, tag="wf")
            wg4 = res.tile([128, 4], bf16, tag="wg4")
            bqc = res.tile([128, 2], f32, tag="bqc")
            bkc = res.tile([128, 2], f32, tag="bkc")
            vb4 = res.tile([128, C], f32, tag="vb4")
            bgneg = res.tile([128, 1], f32, tag="bgneg")

            def wsl(w, dc, cb):
                return w[:, dc * C + cb * 128: dc * C + (cb + 1) * 128]

            # ---------------- DMA emission (priority order) ----------------
            nc.sync.dma_start(wq[:], wq_d[:])
            nc.sync.dma_start(wk[:], wk_d[:])
            nc.sync.dma_start(wg4[:], wg_d[:])
            nc.sync.dma_start(bqc[:], bq_d[:])
            nc.sync.dma_start(bkc[:], bk_d[:])
            nc.sync.dma_start(bgneg[:], bg_d[:])
            for i in range(ND):   # qT a0 halves first (QT[0]a0 chase)
                nc.sync.dma_start(qT[i][:, 0:AW], qT_d[i * 128:(i + 1) * 128, 0:AW])
            for i in range(ND):   # kT full (KT[0] chase)
                nc.sync.dma_start(kT[i][:], kT_d[i * 128:(i + 1) * 128, :])
            nc.sync.dma_start(wv[:], wv_d[:])
            nc.sync.dma_start(vb4[:], bv_d[:])
            for i in range(ND):   # vT (V projection waves chase this)
                nc.sync.dma_start(vT[i][:], vT_d[i * 128:(i + 1) * 128, :])
            nc.sync.dma_start(wf[:], wf_d[:])
            for i in range(ND):   # qT a1 halves
                nc.sync.dma_start(qT[i][:, AW:S], qT_d[i * 128:(i + 1) * 128, AW:S])

            # ---------------- activation / state tiles ----------------
            QT = [acts.tile([128, S], bf16, tag=f"QT{i}", name=f"QT{i}") for i in range(2)]
            KT = [acts.tile([128, S], bf16, tag=f"KT{i}", name=f"KT{i}") for i in range(2)]
            ctxT = [acts.tile([128, S], bf16, tag=f"ctxT{i}", name=f"ctxT{i}")
                    for i in range(2)]
            gscT = [acts.tile([128, HPC], f32, tag=f"gsc{i}", name=f"gsc{i}")
                    for i in range(NSB)]
            V = [acts.tile([128, HPC * 2 * DH], bf16, tag=f"V{i}", name=f"V{i}")
                 for i in range(NSB)]

            # ---------------- pre-phase: chase DMAs ----------------
            # QT[0] a0-half: dc-outer over 2 psum chunks (tags st0, st1)
            qps = [psum.tile([128, 512], f32, tag=f"st{ch}", name=f"qps{ch}")
                   for ch in range(2)]
            for dc in range(ND):
                for ch in range(2):
                    nc.tensor.matmul(
                        qps[ch][:], wsl(wq, dc, 0), qT[dc][:, ch * 512:(ch + 1) * 512],
                        start=(dc == 0), stop=(dc == ND - 1), skip_group_check=True)
            for ch in range(2):
                nc.vector.tensor_scalar_add(
                    QT[0][:, ch * 512:(ch + 1) * 512], qps[ch][:], bqc[:, 0:1])

            # KT[0]: dc-outer over 4 psum chunks (tags st0, st1, ot, fv)
            ktags = ["st0", "st1", "ot", "fv"]
            kps = [psum.tile([128, 512], f32, tag=ktags[a], name=f"kps{a}")
                   for a in range(4)]
            for dc in range(ND):
                for a in range(4):
                    nc.tensor.matmul(
                        kps[a][:], wsl(wk, dc, 0), kT[dc][:, a * 512:(a + 1) * 512],
                        start=(dc == 0), stop=(dc == ND - 1), skip_group_check=True)
            for a in range(4):
                nc.vector.tensor_scalar_add(
                    KT[0][:, a * 512:(a + 1) * 512], kps[a][:], bkc[:, 0:1])

            # gates: gsc = (1 + 1/(1+e^-z))/8, z = K.wg + bg (per head, k-token)
            def emit_gates(sb, cb):
                gps = psum.tile([128, 2], f32, tag="fp", name=f"gps{cb}_{sb}")
                nc.tensor.matmul(
                    gps[:], KT[cb][:, sb * 128:(sb + 1) * 128],
                    wg4[:, 2 * cb: 2 * cb + 2],
                    start=True, stop=True, skip_group_check=True)
                en = work.tile([128, 2], f32, tag="gen", name=f"gen{cb}_{sb}", bufs=2)
                nc.scalar.activation(en[:], gps[:], Act.Exp, bias=bgneg[:], scale=-1.0)
                u = work.tile([128, 2], f32, tag="gu", name=f"gu{cb}_{sb}", bufs=2)
                nc.vector.tensor_scalar_add(u[:], en[:], 1.0)
                r = work.tile([128, 2], f32, tag="gr", name=f"gr{cb}_{sb}", bufs=2)
                nc.vector.reciprocal(r[:], u[:])
                nc.vector.tensor_scalar(
                    gscT[sb][:, 2 * cb: 2 * cb + 2], r[:], 1.0, 0.125,
                    Alu.add, Alu.mult)

            for sb in range(NSB):
                emit_gates(sb, 0)

            # KT[1] (a-serial on fp) + gates for heads 2,3 + QT[1]a0:
            # bridges the PE between KT[0] and the vT DMA landing.
            def proj_chunk_now(w, cbw, x, dest, bias, a):
                ps = psum.tile([128, 512], f32, tag="fp", name=f"pp{id(w)}_{a}")
                for dc in range(ND):
                    nc.tensor.matmul(
                        ps[:], wsl(w, dc, cbw), x[dc][:, a * 512:(a + 1) * 512],
                        start=(dc == 0), stop=(dc == ND - 1), skip_group_check=True)
                nc.vector.tensor_scalar_add(
                    dest[:, a * 512:(a + 1) * 512], ps[:], bias)

            for a in range(4):
                proj_chunk_now(wk, 1, kT, KT[1], bkc[:, 1:2], a)
                for sb in range(4 * a, 4 * a + 4):
                    emit_gates(sb, 1)
            for ch in range(2):
                proj_chunk_now(wq, 1, qT, QT[1], bqc[:, 1:2], ch)

            # bridge dummies: keep the PE p-state hot until vT lands
            for _ in range(40):
                wd = psum.tile([128, 128], f32, tag="fp", name="wdp")
                nc.tensor.matmul(wd[:], wq[:, 0:128], wq[:, 128:256],
                                 start=True, stop=True, skip_group_check=True)

            # V projection: 4 waves of 4 s-blocks, dc-outer across 4 psum
            # banks (psum start=True zeroes whole 2KB banks, so one
            # accumulation group per bank), chasing the vT DMAs.
            for wave in range(4):
                vps = [psum.tile([128, C], f32, tag=ktags[i], name=f"vps{wave}{i}")
                       for i in range(4)]
                for dc in range(ND):
                    for i in range(4):
                        sb = wave * 4 + i
                        nc.tensor.matmul(
                            vps[i][:],
                            vT[dc][:, sb * 128:(sb + 1) * 128],
                            wv[:, dc * C:(dc + 1) * C],
                            start=(dc == 0), stop=(dc == ND - 1),
                            skip_group_check=True)
                for i in range(4):
                    sb = wave * 4 + i
                    vv = V[sb][:].rearrange("p (h x) -> p h x", h=HPC)
                    nc.vector.memset(vv[:, :, DH:2 * DH], 1.0)
                    nc.vector.tensor_tensor(
                        vv[:, :, 0:DH],
                        vps[i][:].rearrange("p (h x) -> p h x", h=HPC),
                        vb4[:].rearrange("p (h x) -> p h x", h=HPC),
                        Alu.add)

            # ---------------- filler units ----------------
            fillers = []   # (cost_cycles, not_before_iter, closure)

            def proj_chunk_units(w, cbw, x, dest, bias, a, nb):
                # one 512-wide projection chunk as 2 units (4 mms each + cast)
                qs = slice(a * 512, (a + 1) * 512)
                ps_box = [None]

                def half(first):
                    if first:
                        ps_box[0] = psum.tile([128, 512], f32, tag="fp",
                                              name=f"fps{len(fillers)}_{a}")
                    for dc in (range(0, 4) if first else range(4, ND)):
                        nc.tensor.matmul(
                            ps_box[0][:], wsl(w, dc, cbw), x[dc][:, qs],
                            start=(dc == 0), stop=(dc == ND - 1),
                            skip_group_check=True)
                    if not first:
                        nc.vector.tensor_scalar_add(dest[:, qs], ps_box[0][:], bias)

                fillers.append((2048, nb, lambda: half(True)))
                fillers.append((2048, nb, lambda: half(False)))

            def outproj_unit(qb, oc, ob, nb):
                def run():
                    ps = psum.tile([128, 512], f32, tag="fp", name=f"ops{qb}_{oc}")
                    for cc in range(2):
                        nc.tensor.matmul(
                            ps[:], ctxT[cc][:, qb * 128:(qb + 1) * 128],
                            wf[:, cc * D + oc * 512: cc * D + (oc + 1) * 512],
                            start=(cc == 0), stop=(cc == 1),
                            skip_group_check=True)
                    nc.vector.tensor_copy(ob[:, oc * 512:(oc + 1) * 512], ps[:])
                    if oc == 1:
                        nc.sync.dma_start(out_d[qb * 128:(qb + 1) * 128, :], ob[:])

                fillers.append((1024, nb, run))

            # QT[0] a1 and QT[1] a1 (qT a1-halves resident by stream start)
            for ch in range(2, 4):
                proj_chunk_units(wq, 0, qT, QT[0], bqc[:, 0:1], ch, 0)
            for ch in range(2, 4):
                proj_chunk_units(wq, 1, qT, QT[1], bqc[:, 1:2], ch, 0)
            # out-projection first q-half (ctxT a0 complete after iter 96)
            obs = {}
            for qb in range(8):
                obs[qb] = outp.tile([128, D], bf16, tag="ob", name=f"ob{qb}")
                outproj_unit(qb, 0, obs[qb], 104)
                outproj_unit(qb, 1, obs[qb], 104)

            # ---------------- attention stream ----------------
            blocks = [(0, 0), (1, 0), (0, 1), (1, 1), (2, 0), (3, 0), (2, 1), (3, 1)]
            budget = [0.0]
            it = [0]

            def pop_fillers():
                popped = 0
                idx = 0
                while idx < len(fillers) and popped < 2:
                    cost, nb, fn = fillers[idx]
                    if it[0] >= nb and budget[0] >= cost:
                        fillers.pop(idx)
                        fn()
                        budget[0] -= cost
                        popped += 1
                        idx = 0
                    elif it[0] >= nb:
                        break   # head eligible but budget exhausted
                    else:
                        idx += 1
                return popped

            def dummy_warm():
                wd = psum.tile([128, 128], f32, tag="fv", name="warm")
                nc.tensor.matmul(wd[:], wq[:, 0:128], wq[:, 128:256],
                                 start=True, stop=True, skip_group_check=True)

            dve_fillers = []

            def defer_normalize(otc, cb, po, a):
                for c in range(4):
                    def chunk(c=c, otc=otc, cb=cb, po=po, a=a):
                        cs = slice(c * 256, (c + 1) * 256)
                        rec = work.tile([DH, 256], f32, tag="rec",
                                        name=f"rc{cb}{po}{a}{c}", bufs=4)
                        nc.vector.reciprocal(rec[:], otc[DH:2 * DH, cs])
                        nc.vector.tensor_tensor(
                            ctxT[cb][po:po + 64,
                                     a * AW + c * 256: a * AW + (c + 1) * 256],
                            otc[0:DH, cs], rec[:], Alu.mult)
                    dve_fillers.append(chunk)

            pts = {}
            for h, a in blocks:
                cb, po = h // 2, (h % 2) * 64
                vsl = slice(h * 2 * DH, (h + 1) * 2 * DH)
                ot = psum.tile([128, AW], f32, tag="ot", name=f"ot{h}{a}")
                for kb in range(NSB):
                    st = psum.tile([128, AW], f32, tag=f"st{kb % 2}",
                                   name=f"st{h}{a}{kb}")
                    for qc in range(2):
                        nc.tensor.matmul(
                            st[:, qc * 512:(qc + 1) * 512],
                            KT[cb][po:po + 64, kb * 128:(kb + 1) * 128],
                            QT[cb][po:po + 64, a * AW + qc * 512: a * AW + (qc + 1) * 512],
                            start=True, stop=True)
                    pt = work.tile([128, AW], bf16, tag="pt",
                                   name=f"pt{h}{a}{kb}", bufs=NPT)
                    nc.scalar.activation(pt[:], st[:], Act.Exp,
                                         scale=gscT[kb][:, h:h + 1])
                    pts[(h, a, kb)] = pt
                    budget[0] = min(budget[0] + 700, 2200)
                    n = pop_fillers()
                    for _ in range(1 if n else 2):
                        dummy_warm()
                    if dve_fillers:
                        dve_fillers.pop(0)()
                    if kb > 0:
                        ptp = pts.pop((h, a, kb - 1))
                        for qc in range(2):
                            nc.tensor.matmul(
                                ot[:, qc * 512:(qc + 1) * 512], V[kb - 1][:, vsl],
                                ptp[:, qc * 512:(qc + 1) * 512],
                                start=(kb == 1), stop=False, skip_group_check=True)
                    it[0] += 1
                ptp = pts.pop((h, a, NSB - 1))
                for qc in range(2):
                    nc.tensor.matmul(
                        ot[:, qc * 512:(qc + 1) * 512], V[NSB - 1][:, vsl],
                        ptp[:, qc * 512:(qc + 1) * 512],
                        start=False, stop=True, skip_group_check=True)
                # release ot with a cheap copy; defer the reciprocal+mult
                # (6.5us DVE chain) off the block-boundary critical path
                otc = work.tile([128, AW], f32, tag="otc", name=f"otc{h}{a}", bufs=2)
                nc.vector.tensor_copy(otc[:], ot[:])
                defer_normalize(otc, cb, po, a)

            # drain deferred normalize chunks, then remaining fillers
            while dve_fillers:
                dve_fillers.pop(0)()
            budget[0] = 1e9
            it[0] = 10 ** 6
            while fillers:
                if pop_fillers() == 0:
                    break

            # ---------------- tail: out-projection q-half 2 ----------------
            for qb in range(8, NSB):
                ob = outp.tile([128, D], bf16, tag="ob", name=f"ob{qb}")
                for oc in range(2):
                    ps = psum.tile([128, 512], f32, tag=("fp", "fv")[oc],
                                   name=f"tps{qb}_{oc}")
                    for cc in range(2):
                        nc.tensor.matmul(
                            ps[:], ctxT[cc][:, qb * 128:(qb + 1) * 128],
                            wf[:, cc * D + oc * 512: cc * D + (oc + 1) * 512],
                            start=(cc == 0), stop=(cc == 1),
                            skip_group_check=True)
                    if oc == 0:
                        nc.vector.tensor_copy(ob[:, 0:512], ps[:])
                    else:
                        nc.scalar.activation(ob[:, 512:1024], ps[:], Act.Copy)
                nc.sync.dma_start(out_d[qb * 128:(qb + 1) * 128, :], ob[:])

    nc.finalize()
    return nc


def get_nc():
    if "nc" not in _nc_cache:
        _nc_cache["nc"] = build_bass()
    return _nc_cache["nc"]


def make_in_maps(query, key_, value, Wq, bq, Wk, bk, Wv, bv, wg, bg, Wo, bo, Wd, bd, Wh, bh):
    """Host-side sharding: returns (in_maps for 8 cores, fused bias)."""
    f = np.asarray
    Wf = f(Wo, np.float64) @ f(Wd, np.float64) @ f(Wh, np.float64)
    bf = (f(bo, np.float64) @ f(Wd, np.float64) @ f(Wh, np.float64)
          + f(bd, np.float64) @ f(Wh, np.float64) + f(bh, np.float64))

    # gnosis weight packed per cb-chunk: col cb*2+hh has wg in rows of
    # head hh within chunk cb
    wgp = np.zeros((128, 4), np.float32)
    wgf = np.asarray(wg, np.float32)
    for cbi in range(2):
        for hh in range(2):
            wgp[hh * 64:(hh + 1) * 64, cbi * 2 + hh] = wgf
    wgp = wgp.astype(BF16)
    bgneg = np.full((128, 1), -np.float32(bg), np.float32)

    def pack_w(Wm, cols):
        # [D, 256] -> [128, ND*256]: w[p, dc*256+c] = Wm[dc*128+p, c]
        Wc = np.asarray(Wm, np.float32)[:, cols]
        return np.ascontiguousarray(
            Wc.reshape(ND, 128, C).transpose(1, 0, 2).reshape(128, ND * C)
        ).astype(BF16)

    xT = []
    for b in range(B):
        xT.append(tuple(
            np.ascontiguousarray(np.asarray(x[b], np.float32).T).astype(BF16)
            for x in (query, key_, value)
        ))

    in_maps = []
    for c in range(NCORES):
        b, g = divmod(c, HPC)
        cols = slice(g * C, (g + 1) * C)
        qTb, kTb, vTb = xT[b]
        wfp = np.ascontiguousarray(Wf[cols, :]).astype(np.float32)
        wfp = np.ascontiguousarray(
            wfp.reshape(2, 128, D).transpose(1, 0, 2).reshape(128, 2 * D)
        ).astype(BF16)
        bqcol = np.zeros((128, 2), np.float32)
        bkcol = np.zeros((128, 2), np.float32)
        for cbi in range(2):
            bqcol[:, cbi] = np.asarray(bq, np.float32)[g * C + cbi * 128:
                                                       g * C + (cbi + 1) * 128]
            bkcol[:, cbi] = np.asarray(bk, np.float32)[g * C + cbi * 128:
                                                       g * C + (cbi + 1) * 128]
        vb4 = np.broadcast_to(
            np.asarray(bv, np.float32)[cols][None, :], (128, C)).copy()
        in_maps.append({
            "qT": qTb, "kT": kTb, "vT": vTb,
            "wqp": pack_w(Wq, cols),
            "wkp": pack_w(Wk, cols),
            "wvp": pack_w(Wv, cols),
            "wfp": wfp,
            "wgp": wgp, "bgneg": bgneg,
            "bqc": bqcol, "bkc": bkcol, "vb4": vb4,
        })
    return in_maps, bf.astype(np.float32)


def gather(results, bf):
    out = np.zeros((B, S, D), np.float32)
    for c in range(NCORES):
        b = c // HPC
        out[b] += np.asarray(results[c]["out"], np.float32)
    out += bf[None, None, :]
    return out


def kernel(**inputs):
    from concourse.bass_utils import run_bass_kernel_spmd

    nc = get_nc()
    in_maps, bf = make_in_maps(**inputs)
    res = run_bass_kernel_spmd(nc, in_maps, core_ids=list(range(NCORES)))
    return gather(res.results, bf)


# revision 14
# speedup vs baseline: 1.2216x; 1.0997x over previous
"""Trainium2 Bass kernel for MultiHeadSyntonicAttention.

Problem: B=2, S=2048, D=1024, H=16 heads, DH=64.
  q/k/v = Linear(query/key/value); per-head gnosis gate
  gn = sigmoid(k . wg + bg); scores = (q k^T / sqrt(dh)) * (1+gn);
  out = softmax(scores) v;  out = ((out Wo+bo) Wd+bd) Wh+bh.

Sharding (8 cores): core c -> batch b=c//4, head-group g=c%4 (4 heads).
Each core computes its heads' attention and a row-slice partial of the
fused output projection Wf = Wo@Wd@Wh; host sums 4 partials per batch.

Schedule design (v2): the Act engine is a hard serial floor (128 exp
instructions of [128,1024] ~ 154us measured) and the PE only reaches
its 2.4GHz p-state when continuously busy, so the kernel is built as a
gapless act-stream of exps fed by a PE stream that never idles:
  - pre-phase "chases" the input DMAs (QT[0] a0-half, KT[0], then the
    full V projection in 2 waves of 8 s-blocks) with dc-outer
    accumulation across psum banks; gates for heads 0,1 right after
    KT[0].
  - attention stream (h0a0,h1a0,h0a1,h1a1,h2a0,h3a0,h2a1,h3a1) emits
    QK -> exp(scale=(1+gn)/8 per k-row) -> lag-1 PV per k-block, and
    pops filler PE work between QK and PV: KT[1]+gates(2,3), QT chunks,
    and the fused output projection for q-blocks 0-7 near the end.
    When the filler queue is dry an ldweights-only warmup keeps the PE
    p-state hot (touches no psum).
  - biases ride the psum->sbuf casts (DVE tensor_scalar) instead of
    rank-1 matmuls; softmax normalization divides via
    reciprocal_approx_fast directly out of the ot psum; the gnosis
    sigmoid is 1/(1+e^-z) computed with the Exp table so the act engine
    never reloads tables mid-stream.
PSUM tags: st0(2) st1(2) ot(2) fv(1) fp(1) banks = 8.
"""

import sys

sys.path.insert(0, "/opt/trn_rl_repo")

import numpy as np
import ml_dtypes

BF16 = ml_dtypes.bfloat16

B, S, D, H = 2, 2048, 1024, 16
DH = D // H          # 64
HPC = 4              # heads per core
C = HPC * DH         # 256 head-local columns per core
NCORES = 8
ND = D // 128        # 8 d-chunks
NSB = S // 128       # 16 s-blocks
AW = 1024            # attention q-tile width (2 PSUM banks)
NA = S // AW         # 2
NPT = 18             # pt ring depth (PV lags 16)

_nc_cache = {}


def build_bass():
    import concourse.bass as bass
    import concourse.mybir as mybir
    import concourse.tile as tile
    from concourse import bacc

    f32 = mybir.dt.float32
    bf16 = mybir.dt.bfloat16
    Alu = mybir.AluOpType
    Act = mybir.ActivationFunctionType

    nc = bacc.Bacc(None, target_bir_lowering=False, name="syntonic_attn")

    qT_d = nc.dram_tensor("qT", [D, S], bf16, kind="ExternalInput")
    kT_d = nc.dram_tensor("kT", [D, S], bf16, kind="ExternalInput")
    vT_d = nc.dram_tensor("vT", [D, S], bf16, kind="ExternalInput")
    # weights pre-packed host-side into tile layout
    wq_d = nc.dram_tensor("wqp", [128, ND * C], bf16, kind="ExternalInput")
    wk_d = nc.dram_tensor("wkp", [128, ND * C], bf16, kind="ExternalInput")
    wv_d = nc.dram_tensor("wvp", [128, ND * C], bf16, kind="ExternalInput")
    wf_d = nc.dram_tensor("wfp", [128, 2 * D], bf16, kind="ExternalInput")
    wg_d = nc.dram_tensor("wgp", [128, 2 * 2], bf16, kind="ExternalInput")
    bq_d = nc.dram_tensor("bqc", [128, 2], f32, kind="ExternalInput")
    bk_d = nc.dram_tensor("bkc", [128, 2], f32, kind="ExternalInput")
    bv_d = nc.dram_tensor("vb4", [128, C], f32, kind="ExternalInput")
    bg_d = nc.dram_tensor("bgneg", [128, 1], f32, kind="ExternalInput")
    out_d = nc.dram_tensor("out", [S, D], bf16, kind="ExternalOutput")

    with tile.TileContext(nc) as tc:
        with (
            tc.tile_pool(name="res", bufs=1) as res,
            tc.tile_pool(name="acts", bufs=1) as acts,
            tc.tile_pool(name="work", bufs=2) as work,
            tc.tile_pool(name="outp", bufs=3) as outp,
            tc.tile_pool(name="psum", bufs=1, space="PSUM") as psum,
        ):
            # ---------------- resident input tiles ----------------
            qT = [res.tile([128, S], bf16, tag=f"qT{i}", name=f"qT{i}") for i in range(ND)]
            kT = [res.tile([128, S], bf16, tag=f"kT{i}", name=f"kT{i}") for i in range(ND)]
            vT = [res.tile([128, S], bf16, tag=f"vT{i}", name=f"vT{i}") for i in range(ND)]
            wq = res.tile([128, ND * C], bf16, tag="wq")
            wk = res.tile([128, ND * C], bf16, tag="wk")
            wv = res.tile([128, ND * C], bf16, tag="wv")
            wf = res.tile([128, 2 * D], bf16, tag="wf")
            wg4 = res.tile([128, 4], bf16, tag="wg4")
            bqc = res.tile([128, 2], f32, tag="bqc")
            bkc = res.tile([128, 2], f32, tag="bkc")
            vb4 = res.tile([128, C], f32, tag="vb4")
            bgneg = res.tile([128, 1], f32, tag="bgneg")

            def wsl(w, dc, cb):
                return w[:, dc * C + cb * 128: dc * C + (cb + 1) * 128]

            # ---------------- DMA emission (priority order) ----------------
            nc.sync.dma_start(wq[:], wq_d[:])
            nc.sync.dma_start(wk[:], wk_d[:])
            nc.sync.dma_start(wg4[:], wg_d[:])
            nc.sync.dma_start(bqc[:], bq_d[:])
            nc.sync.dma_start(bkc[:], bk_d[:])
            nc.sync.dma_start(bgneg[:], bg_d[:])
            for i in range(ND):   # qT a0 halves first (QT[0]a0 chase)
                nc.sync.dma_start(qT[i][:, 0:AW], qT_d[i * 128:(i + 1) * 128, 0:AW])
            for i in range(ND):   # kT full (KT[0] chase)
                nc.sync.dma_start(kT[i][:], kT_d[i * 128:(i + 1) * 128, :])
            nc.sync.dma_start(wv[:], wv_d[:])
            nc.sync.dma_start(vb4[:], bv_d[:])
            for i in range(ND):   # vT (V projection waves chase this)
                nc.sync.dma_start(vT[i][:], vT_d[i * 128:(i + 1) * 128, :])
            nc.sync.dma_start(wf[:], wf_d[:])
            for i in range(ND):   # qT a1 halves
                nc.sync.dma_start(qT[i][:, AW:S], qT_d[i * 128:(i + 1) * 128, AW:S])

            # ---------------- activation / state tiles ----------------
            QT = [acts.tile([128, S], bf16, tag=f"QT{i}", name=f"QT{i}") for i in range(2)]
            KT = [acts.tile([128, S], bf16, tag=f"KT{i}", name=f"KT{i}") for i in range(2)]
            ctxT = [acts.tile([128, S], bf16, tag=f"ctxT{i}", name=f"ctxT{i}")
                    for i in range(2)]
            gscT = [acts.tile([128, HPC], f32, tag=f"gsc{i}", name=f"gsc{i}")
                    for i in range(NSB)]
            V = [acts.tile([128, HPC * 2 * DH], bf16, tag=f"V{i}", name=f"V{i}")
                 for i in range(NSB)]

            # ---------------- pre-phase: chase DMAs ----------------
            # QT[0] a0-half: dc-outer over 2 psum chunks (tags st0, st1)
            qps = [psum.tile([128, 512], f32, tag=f"st{ch}", name=f"qps{ch}")
                   for ch in range(2)]
            for dc in range(ND):
                for ch in range(2):
                    nc.tensor.matmul(
                        qps[ch][:], wsl(wq, dc, 0), qT[dc][:, ch * 512:(ch + 1) * 512],
                        start=(dc == 0), stop=(dc == ND - 1), skip_group_check=True)
            for ch in range(2):
                nc.vector.tensor_scalar_add(
                    QT[0][:, ch * 512:(ch + 1) * 512], qps[ch][:], bqc[:, 0:1])

            # KT[0]: dc-outer over 4 psum chunks (tags st0, st1, ot, fv)
            ktags = ["st0", "st1", "ot", "fv"]
            kps = [psum.tile([128, 512], f32, tag=ktags[a], name=f"kps{a}")
                   for a in range(4)]
            for dc in range(ND):
                for a in range(4):
                    nc.tensor.matmul(
                        kps[a][:], wsl(wk, dc, 0), kT[dc][:, a * 512:(a + 1) * 512],
                        start=(dc == 0), stop=(dc == ND - 1), skip_group_check=True)
            for a in range(4):
                nc.vector.tensor_scalar_add(
                    KT[0][:, a * 512:(a + 1) * 512], kps[a][:], bkc[:, 0:1])

            # gates: gsc = (1 + 1/(1+e^-z))/8, z = K.wg + bg (per head, k-token)
            def emit_gates(sb, cb):
                gps = psum.tile([128, 2], f32, tag="fp", name=f"gps{cb}_{sb}")
                nc.tensor.matmul(
                    gps[:], KT[cb][:, sb * 128:(sb + 1) * 128],
                    wg4[:, 2 * cb: 2 * cb + 2],
                    start=True, stop=True, skip_group_check=True)
                en = work.tile([128, 2], f32, tag="gen", name=f"gen{cb}_{sb}", bufs=2)
                nc.scalar.activation(en[:], gps[:], Act.Exp, bias=bgneg[:], scale=-1.0)
                u = work.tile([128, 2], f32, tag="gu", name=f"gu{cb}_{sb}", bufs=2)
                nc.vector.tensor_scalar_add(u[:], en[:], 1.0)
                r = work.tile([128, 2], f32, tag="gr", name=f"gr{cb}_{sb}", bufs=2)
                nc.vector.reciprocal(r[:], u[:])
                nc.vector.tensor_scalar(
                    gscT[sb][:, 2 * cb: 2 * cb + 2], r[:], 1.0, 0.125,
                    Alu.add, Alu.mult)

            for sb in range(NSB):
                emit_gates(sb, 0)

            # ---------------- filler units ----------------
            fillers = []   # (cost_cycles, not_before_iter, closure)

            def proj_chunk_units(w, cbw, x, dest, bias, a, nb):
                # one 512-wide projection chunk as 2 units (4 mms each + cast)
                qs = slice(a * 512, (a + 1) * 512)
                ps_box = [None]

                def half(first):
                    if first:
                        ps_box[0] = psum.tile([128, 512], f32, tag="fp",
                                              name=f"fps{len(fillers)}_{a}")
                    for dc in (range(0, 4) if first else range(4, ND)):
                        nc.tensor.matmul(
                            ps_box[0][:], wsl(w, dc, cbw), x[dc][:, qs],
                            start=(dc == 0), stop=(dc == ND - 1),
                            skip_group_check=True)
                    if not first:
                        nc.vector.tensor_scalar_add(dest[:, qs], ps_box[0][:], bias)

                fillers.append((2048, nb, lambda: half(True)))
                fillers.append((2048, nb, lambda: half(False)))

            def outproj_unit(qb, oc, ob, nb):
                def run():
                    ps = psum.tile([128, 512], f32, tag="fp", name=f"ops{qb}_{oc}")
                    for cc in range(2):
                        nc.tensor.matmul(
                            ps[:], ctxT[cc][:, qb * 128:(qb + 1) * 128],
                            wf[:, cc * D + oc * 512: cc * D + (oc + 1) * 512],
                            start=(cc == 0), stop=(cc == 1),
                            skip_group_check=True)
                    nc.vector.tensor_copy(ob[:, oc * 512:(oc + 1) * 512], ps[:])
                    if oc == 1:
                        nc.sync.dma_start(out_d[qb * 128:(qb + 1) * 128, :], ob[:])

                fillers.append((1024, nb, run))

            # V projection units: self-contained (8 mm + memset + biased
            # cast) on the fv bank; eligible once vT has landed (~iter 13),
            # one per iteration, 3 ahead of the lag-16 PV pipeline.
            def v_unit(sb, nb):
                def run():
                    ps = psum.tile([128, C], f32, tag="fv", name=f"vps{sb}")
                    for dc in range(ND):
                        nc.tensor.matmul(
                            ps[:], vT[dc][:, sb * 128:(sb + 1) * 128],
                            wv[:, dc * C:(dc + 1) * C],
                            start=(dc == 0), stop=(dc == ND - 1),
                            skip_group_check=True)
                    vv = V[sb][:].rearrange("p (h x) -> p h x", h=HPC)
                    nc.vector.memset(vv[:, :, DH:2 * DH], 1.0)
                    nc.vector.tensor_tensor(
                        vv[:, :, 0:DH],
                        ps[:].rearrange("p (h x) -> p h x", h=HPC),
                        vb4[:].rearrange("p (h x) -> p h x", h=HPC),
                        Alu.add)
                fillers.append((0, nb, run))

            for sb in range(NSB):
                v_unit(sb, 13 + sb)
            # QT[0] a1 first (needed by iter 32; qT a1 lands ~iter 8)
            for ch in range(2, 4):
                proj_chunk_units(wq, 0, qT, QT[0], bqc[:, 0:1], ch, 8)
            # KT[1] + gates for heads 2,3 + QT[1] a0 (needed by iter 64)
            for a in range(4):
                proj_chunk_units(wk, 1, kT, KT[1], bkc[:, 1:2], a, 0)
                for sb in range(4 * a, 4 * a + 4):
                    fillers.append((128, 0, (lambda s=sb: emit_gates(s, 1))))
            for ch in range(2):
                proj_chunk_units(wq, 1, qT, QT[1], bqc[:, 1:2], ch, 0)
            # QT[1] a1 (needed by iter 96)
            for ch in range(2, 4):
                proj_chunk_units(wq, 1, qT, QT[1], bqc[:, 1:2], ch, 8)

            # ---------------- attention stream ----------------
            blocks = [(0, 0), (1, 0), (0, 1), (1, 1), (2, 0), (3, 0), (2, 1), (3, 1)]
            budget = [0.0]
            it = [0]

            def pop_fillers():
                popped = 0
                idx = 0
                while idx < len(fillers) and popped < 2:
                    cost, nb, fn = fillers[idx]
                    if it[0] >= nb and budget[0] >= cost:
                        fillers.pop(idx)
                        fn()
                        budget[0] -= cost
                        popped += 1
                        idx = 0
                    elif it[0] >= nb:
                        break   # head eligible but budget exhausted
                    else:
                        idx += 1
                return popped

            def dummy_warm():
                wd = psum.tile([128, 128], f32, tag="fv", name="warm")
                nc.tensor.matmul(wd[:], wq[:, 0:128], wq[:, 128:256],
                                 start=True, stop=True, skip_group_check=True)

            dve_fillers = []

            def defer_normalize(otc, cb, po, a):
                for c in range(4):
                    def chunk(c=c, otc=otc, cb=cb, po=po, a=a):
                        cs = slice(c * 256, (c + 1) * 256)
                        rec = work.tile([DH, 256], f32, tag="rec",
                                        name=f"rc{cb}{po}{a}{c}", bufs=4)
                        nc.vector.reciprocal(rec[:], otc[DH:2 * DH, cs])
                        nc.vector.tensor_tensor(
                            ctxT[cb][po:po + 64,
                                     a * AW + c * 256: a * AW + (c + 1) * 256],
                            otc[0:DH, cs], rec[:], Alu.mult)
                    dve_fillers.append(chunk)

            PVLAG = 16
            pts = {}
            ots = {}

            def emit_pv(j):
                # j-th global (block, kb) pair; start/stop per 16-group
                jb, kbj = j // 16, j % 16
                h, a = blocks[jb]
                cb, po = h // 2, (h % 2) * 64
                vsl = slice(h * 2 * DH, (h + 1) * 2 * DH)
                if kbj == 0:
                    ots[jb] = psum.tile([128, AW], f32, tag="ot", name=f"ot{jb}")
                ptp = pts.pop(j)
                for qc in range(2):
                    nc.tensor.matmul(
                        ots[jb][:, qc * 512:(qc + 1) * 512], V[kbj][:, vsl],
                        ptp[:, qc * 512:(qc + 1) * 512],
                        start=(kbj == 0), stop=(kbj == 15),
                        skip_group_check=True)
                if kbj == 15:
                    ot = ots.pop(jb)
                    otc = work.tile([128, AW], f32, tag="otc",
                                    name=f"otc{jb}", bufs=2)
                    nc.vector.tensor_copy(otc[:], ot[:])
                    defer_normalize(otc, cb, po, a)

            for i in range(8 * NSB):
                jb, kb = i // 16, i % 16
                h, a = blocks[jb]
                cb, po = h // 2, (h % 2) * 64
                st = psum.tile([128, AW], f32, tag=f"st{i % 2}",
                               name=f"st{jb}_{kb}")
                for qc in range(2):
                    nc.tensor.matmul(
                        st[:, qc * 512:(qc + 1) * 512],
                        KT[cb][po:po + 64, kb * 128:(kb + 1) * 128],
                        QT[cb][po:po + 64, a * AW + qc * 512: a * AW + (qc + 1) * 512],
                        start=True, stop=True)
                pt = work.tile([128, AW], bf16, tag="pt",
                               name=f"pt{jb}_{kb}", bufs=NPT)
                nc.scalar.activation(pt[:], st[:], Act.Exp,
                                     scale=gscT[kb][:, h:h + 1])
                pts[i] = pt
                budget[0] = min(budget[0] + 700, 2200)
                n = pop_fillers()
                for _ in range(1 if n else 2):
                    dummy_warm()
                if dve_fillers:
                    dve_fillers.pop(0)()
                if i >= PVLAG:
                    emit_pv(i - PVLAG)
                it[0] += 1

            # drain the lagged PV pipeline
            for j in range(8 * NSB - PVLAG, 8 * NSB):
                emit_pv(j)

            # drain deferred normalize chunks, then remaining fillers
            while dve_fillers:
                dve_fillers.pop(0)()
            budget[0] = 1e9
            it[0] = 10 ** 6
            while fillers:
                if pop_fillers() == 0:
                    break

            # ---------------- tail: full out-projection ----------------
            for qb in range(NSB):
                ob = outp.tile([128, D], bf16, tag="ob", name=f"ob{qb}")
                for oc in range(2):
                    ps = psum.tile([128, 512], f32, tag=("fp", "fv")[oc],
                                   name=f"tps{qb}_{oc}")
                    for cc in range(2):
                        nc.tensor.matmul(
                            ps[:], ctxT[cc][:, qb * 128:(qb + 1) * 128],
                            wf[:, cc * D + oc * 512: cc * D + (oc + 1) * 512],
                            start=(cc == 0), stop=(cc == 1),
                            skip_group_check=True)
                    if oc == 0:
                        nc.vector.tensor_copy(ob[:, 0:512], ps[:])
                    else:
                        nc.scalar.activation(ob[:, 512:1024], ps[:], Act.Copy)
                nc.sync.dma_start(out_d[qb * 128:(qb + 1) * 128, :], ob[:])

    nc.finalize()
    return nc


def get_nc():
    if "nc" not in _nc_cache:
        _nc_cache["nc"] = build_bass()
    return _nc_cache["nc"]


def make_in_maps(query, key_, value, Wq, bq, Wk, bk, Wv, bv, wg, bg, Wo, bo, Wd, bd, Wh, bh):
    """Host-side sharding: returns (in_maps for 8 cores, fused bias)."""
    f = np.asarray
    Wf = f(Wo, np.float64) @ f(Wd, np.float64) @ f(Wh, np.float64)
    bf = (f(bo, np.float64) @ f(Wd, np.float64) @ f(Wh, np.float64)
          + f(bd, np.float64) @ f(Wh, np.float64) + f(bh, np.float64))

    # gnosis weight packed per cb-chunk: col cb*2+hh has wg in rows of
    # head hh within chunk cb
    wgp = np.zeros((128, 4), np.float32)
    wgf = np.asarray(wg, np.float32)
    for cbi in range(2):
        for hh in range(2):
            wgp[hh * 64:(hh + 1) * 64, cbi * 2 + hh] = wgf
    wgp = wgp.astype(BF16)
    bgneg = np.full((128, 1), -np.float32(bg), np.float32)

    def pack_w(Wm, cols):
        # [D, 256] -> [128, ND*256]: w[p, dc*256+c] = Wm[dc*128+p, c]
        Wc = np.asarray(Wm, np.float32)[:, cols]
        return np.ascontiguousarray(
            Wc.reshape(ND, 128, C).transpose(1, 0, 2).reshape(128, ND * C)
        ).astype(BF16)

    xT = []
    for b in range(B):
        xT.append(tuple(
            np.ascontiguousarray(np.asarray(x[b], np.float32).T).astype(BF16)
            for x in (query, key_, value)
        ))

    in_maps = []
    for c in range(NCORES):
        b, g = divmod(c, HPC)
        cols = slice(g * C, (g + 1) * C)
        qTb, kTb, vTb = xT[b]
        wfp = np.ascontiguousarray(Wf[cols, :]).astype(np.float32)
        wfp = np.ascontiguousarray(
            wfp.reshape(2, 128, D).transpose(1, 0, 2).reshape(128, 2 * D)
        ).astype(BF16)
        bqcol = np.zeros((128, 2), np.float32)
        bkcol = np.zeros((128, 2), np.float32)
        for cbi in range(2):
            bqcol[:, cbi] = np.asarray(bq, np.float32)[g * C + cbi * 128:
                                                       g * C + (cbi + 1) * 128]
            bkcol[:, cbi] = np.asarray(bk, np.float32)[g * C + cbi * 128:
                                                       g * C + (cbi + 1) * 128]
        vb4 = np.broadcast_to(
            np.asarray(bv, np.float32)[cols][None, :], (128, C)).copy()
        in_maps.append({
            "qT": qTb, "kT": kTb, "vT": vTb,
            "wqp": pack_w(Wq, cols),
            "wkp": pack_w(Wk, cols),
            "wvp": pack_w(Wv, cols),
            "wfp": wfp,
            "wgp": wgp, "bgneg": bgneg,
            "bqc": bqcol, "bkc": bkcol, "vb4": vb4,
        })
    return in_maps, bf.astype(np.float32)


def gather(results, bf):
    out = np.zeros((B, S, D), np.float32)
    for c in range(NCORES):
        b = c // HPC
        out[b] += np.asarray(results[c]["out"], np.float32)
    out += bf[None, None, :]
    return out


def kernel(**inputs):
    from concourse.bass_utils import run_bass_kernel_spmd

    nc = get_nc()
    in_maps, bf = make_in_maps(**inputs)
    res = run_bass_kernel_spmd(nc, in_maps, core_ids=list(range(NCORES)))
    return gather(res.results, bf)
